# revision 11
# baseline (speedup 1.0000x reference)
"""2-layer GraphSAGE (mean) over 8 TRN2 NeuronCores.

Strategy:
  - Destination-shard nodes across 8 cores (6250/core). x is replicated into
    every core's DRAM at input-load time, so layer 1 needs no communication.
  - Host (numpy, uncounted) builds per-core padded-CSR gather schedules:
    nodes within a core are sorted by degree and packed into 128-node blocks;
    each block's neighbor lists are padded to the block max (K) and split by
    source-id half so every dma_gather table stays < 32768 rows (int16 idxs).
  - Device per block: dma_gather (transpose mode, bf16) pulls neighbor rows
    as columns [feat(p), slot]; DVE tree-adds fold K slots -> agg; multiply
    by 1/deg; two PE matmuls (mean@W_l + x@W_r) accumulate in PSUM; ACT does
    bias+relu; DVE applies the dropout mask. h rows go to DRAM via a PE
    transpose.
  - One AllGather exchanges h slices between layers; layer 2 gathers from the
    gathered table and writes the output transposed (host untransposes).
"""

import sys

for _p in ("/opt/trn_rl_repo",):
    if _p not in sys.path:
        sys.path.insert(0, _p)

import hashlib
import numpy as np
import ml_dtypes

BF16 = ml_dtypes.bfloat16
P = 128
D = 128


# --------------------------------------------------------------------------
# Host-side schedule construction
# --------------------------------------------------------------------------

def build_meta(src, dst, N, ncores, slot_budget=32):
    """Build the shared (SPMD) gather schedule + per-core index arrays."""
    src = np.asarray(src, np.int64)
    dst = np.asarray(dst, np.int64)
    npc = N // ncores
    assert npc * ncores == N, (N, ncores)
    blocks = -(-npc // P)
    npad = blocks * P
    assert npc < npad, "need pad rows for zero-sentinels in layer-2 table"
    assert ncores % 2 == 0
    split1 = N // 2
    split2 = (ncores // 2) * npad
    assert split1 + 1 <= 32768 and (N - split1) + 1 <= 32768
    assert split2 <= 32768

    deg = np.bincount(dst, minlength=N)
    invcnt = (1.0 / np.maximum(deg, 1.0)).astype(np.float32)

    perm = -np.ones((ncores, npad), np.int64)
    rank = np.empty(N, np.int64)
    for c in range(ncores):
        ids = np.arange(c * npc, (c + 1) * npc)
        order = np.argsort(-deg[ids], kind="stable")
        pids = ids[order]
        perm[c, :npc] = pids
        rank[pids] = np.arange(npc)
    core_of_node = np.arange(N) // npc
    pos = core_of_node * npad + rank  # position in the allgathered h table

    # per (layer, half, core): edge placement (block, lane, k, local idx)
    Ks = np.zeros((2, 2, ncores, blocks), np.int64)
    placed = {}
    dcore = dst // npc
    for c in range(ncores):
        sel = dcore == c
        s_c = src[sel]
        r_all = rank[dst[sel]]
        for L in range(2):
            key = s_c if L == 0 else pos[s_c]
            spl = split1 if L == 0 else split2
            half = (key >= spl).astype(np.int64)
            li = np.where(half == 0, key, key - spl)
            for h in (0, 1):
                m2 = half == h
                rr = r_all[m2]
                ll = li[m2]
                o = np.argsort(rr, kind="stable")
                rr = rr[o]
                ll = ll[o]
                cnts = np.bincount(rr, minlength=npc)
                first = np.concatenate([[0], np.cumsum(cnts)])[:-1]
                k = np.arange(len(rr)) - first[rr]
                blk = rr // P
                lane = rr % P
                Kblk = np.zeros(blocks, np.int64)
                if len(rr):
                    np.maximum.at(Kblk, blk, k + 1)
                placed[(L, h, c)] = (blk, lane, k, ll)
                Ks[L, h, c] = Kblk

    K = np.maximum(Ks.max(axis=2), 1)  # [layer][half][block], shared schedule

    layers = []
    idx_arrays = [[None] * ncores, [None] * ncores]
    for L in range(2):
        KA, KB = K[L, 0], K[L, 1]
        # greedy grouping of blocks under the per-half slot budget
        groups_blocks = []
        cur, curA, curB = [], 0, 0
        for b in range(blocks):
            if cur and (curA + KA[b] > slot_budget or curB + KB[b] > slot_budget):
                groups_blocks.append(cur)
                cur, curA, curB = [], 0, 0
            cur.append(b)
            curA += KA[b]
            curB += KB[b]
        groups_blocks.append(cur)

        sentA = split1 if L == 0 else npc
        sentB = (N - split1) if L == 0 else npc

        # assembly layout: per group, all A segments then all B segments
        total = int((KA.sum() + KB.sum()) * P)
        baseA = np.zeros(blocks, np.int64)  # elem offset of block b's A segment
        baseB = np.zeros(blocks, np.int64)
        groups = []
        off = 0
        for g in groups_blocks:
            ginfo = {"blocks": []}
            a0 = off
            for b in g:
                baseA[b] = off
                off += int(KA[b]) * P
            ginfo["colA"] = a0 // 16
            ginfo["NIA"] = off - a0
            b0 = off
            for b in g:
                baseB[b] = off
                off += int(KB[b]) * P
            ginfo["colB"] = b0 // 16
            ginfo["NIB"] = off - b0
            oa = 0
            ob = 0
            for b in g:
                ginfo["blocks"].append(
                    (b, oa, int(KA[b]), ob, int(KB[b]))
                )
                oa += int(KA[b])
                ob += int(KB[b])
            groups.append(ginfo)
        assert off == total

        for c in range(ncores):
            flat = np.empty(total, np.int16)
            # default sentinels
            for g in groups:
                a0 = g["colA"] * 16
                flat[a0:a0 + g["NIA"]] = sentA
                b0 = g["colB"] * 16
                flat[b0:b0 + g["NIB"]] = sentB
            for h, base in ((0, baseA), (1, baseB)):
                blk, lane, k, ll = placed[(L, h, c)]
                if len(blk):
                    np.add.at  # noqa (no-op; keep linters quiet)
                    posn = base[blk] + k * P + lane
                    flat[posn] = ll.astype(np.int16)
            idx_arrays[L][c] = np.ascontiguousarray(np.tile(flat.reshape(-1, 16).T, (8, 1)))

        layers.append({"groups": groups, "C": total // 16})

    return {
        "N": N, "ncores": ncores, "npc": npc, "blocks": blocks, "npad": npad,
        "split1": split1, "split2": split2,
        "perm": perm, "invcnt": invcnt,
        "layers": layers, "idx": idx_arrays,
        "tabA1": split1 + 1, "tabB1": (N - split1) + 1,
    }


# --------------------------------------------------------------------------
# Bass graph
# --------------------------------------------------------------------------

def build_nc(meta):
    import os
    kstage = os.environ.get("KSTAGE", "5")
    stage = {"g": 1, "m": 1, "a": 1}.get(kstage, int(kstage) if kstage.isdigit() else 5)
    from concourse import bacc, mybir
    from concourse.tile import TileContext
    from concourse.masks import make_identity

    dt = mybir.dt
    ALU = mybir.AluOpType
    AF = mybir.ActivationFunctionType
    m = meta
    npad, ncores, blocks = m["npad"], m["ncores"], m["blocks"]

    nc = bacc.Bacc()

    xtabA = nc.declare_dram_parameter("xtabA", [m["tabA1"], D], dt.bfloat16, isOutput=False)
    xtabB = nc.declare_dram_parameter("xtabB", [m["tabB1"], D], dt.bfloat16, isOutput=False)
    idx1 = nc.declare_dram_parameter("idx1", [P, m["layers"][0]["C"]], dt.int16, isOutput=False)
    idx2 = nc.declare_dram_parameter("idx2", [P, m["layers"][1]["C"]], dt.int16, isOutput=False)
    xT = nc.declare_dram_parameter("xT", [P, npad], dt.bfloat16, isOutput=False)
    m2T = nc.declare_dram_parameter("m2T", [P, npad], dt.bfloat16, isOutput=False)
    invT = nc.declare_dram_parameter("invT", [P, npad], dt.float32, isOutput=False)
    w1l = nc.declare_dram_parameter("w1l", [P, P], dt.bfloat16, isOutput=False)
    w1r = nc.declare_dram_parameter("w1r", [P, P], dt.bfloat16, isOutput=False)
    w2l = nc.declare_dram_parameter("w2l", [P, P], dt.bfloat16, isOutput=False)
    w2r = nc.declare_dram_parameter("w2r", [P, P], dt.bfloat16, isOutput=False)
    b1 = nc.declare_dram_parameter("b1", [P, 1], dt.float32, isOutput=False)
    b2 = nc.declare_dram_parameter("b2", [P, 1], dt.float32, isOutput=False)
    outT = nc.declare_dram_parameter("outT", [P, npad], dt.float32, isOutput=True)

    cc_in = nc.dram_tensor("cc_in", [npad, D], dt.bfloat16)
    h_full = nc.dram_tensor("h_full", [ncores * npad, D], dt.bfloat16, addr_space="Shared")

    def tree(gt, off, K):
        """Fold K slots at slot-offset `off` of gather tile gt down to 1."""
        while K > 1:
            h = K // 2
            a = K - h
            dstap = gt[:, (off) * P:(off + h) * P]
            srcap = gt[:, (off + a) * P:(off + a + h) * P]
            nc.vector.tensor_tensor(out=dstap, in0=dstap, in1=srcap, op=ALU.add)
            K = a
        return gt[:, off * P:(off + 1) * P]

    with TileContext(nc) as tc:
        with (
            tc.tile_pool(name="persist", bufs=1) as pers,
            tc.tile_pool(name="work", bufs=4) as wp,
            tc.tile_pool(name="gath", bufs=2) as gp,
            tc.tile_pool(name="ps", bufs=2, space="PSUM") as psp,
            tc.tile_pool(name="pst", bufs=2, space="PSUM") as psq,
        ):
            def load(dram, shape, dtype, tag):
                t = pers.tile(shape, dtype, tag=tag)
                nc.sync.dma_start(out=t[:], in_=dram[:])
                return t

            idx_sb = [
                load(idx1, [P, m["layers"][0]["C"]], dt.int16, "idx1"),
                load(idx2, [P, m["layers"][1]["C"]], dt.int16, "idx2"),
            ]
            xT_sb = load(xT, [P, npad], dt.bfloat16, "xT")
            m2T_sb = load(m2T, [P, npad], dt.bfloat16, "m2T")
            invT_sb = load(invT, [P, npad], dt.float32, "invT")
            w_sb = [
                (load(w1l, [P, P], dt.bfloat16, "w1l"), load(w1r, [P, P], dt.bfloat16, "w1r")),
                (load(w2l, [P, P], dt.bfloat16, "w2l"), load(w2r, [P, P], dt.bfloat16, "w2r")),
            ]
            b1_sb = load(b1, [P, 1], dt.float32, "b1")
            b2_sb = load(b2, [P, 1], dt.float32, "b2")
            ident = pers.tile([P, P], dt.bfloat16, tag="ident")
            make_identity(nc, ident[:])
            hT_sb = pers.tile([P, npad], dt.bfloat16, tag="hT")

            for L in range(2 if stage >= 4 else 1):
                lm = m["layers"][L]
                if L == 0:
                    tabs = (xtabA[:, :], xtabB[:, :])
                else:
                    tabs = (h_full[0:m["split2"], :], h_full[m["split2"]:, :])
                wl_sb, wr_sb = w_sb[L]
                rhs2 = xT_sb if L == 0 else hT_sb
                for g in lm["groups"]:
                    gts = []
                    for tab, col, NI in (
                        (tabs[0], g["colA"], g["NIA"]),
                        (tabs[1], g["colB"], g["NIB"]),
                    ):
                        t = gp.tile([P, NI], dt.bfloat16, tag=f"g{len(gts)}")
                        nc.gpsimd.dma_gather(
                            out_ap=t[:].rearrange("p (o n) -> p o n", o=1),
                            in_ap=tab,
                            idxs_ap=idx_sb[L][:, col:col + NI // 16],
                            num_idxs=NI,
                            num_idxs_reg=NI,
                            elem_size=D,
                            transpose=True,
                            single_packet=False,
                        )
                        gts.append(t)
                    for b, oA, KA, oB, KB in g["blocks"]:
                        blk = slice(b * P, (b + 1) * P)
                        slabA = tree(gts[0], oA, KA)
                        slabB = tree(gts[1], oB, KB)
                        agg = wp.tile([P, P], dt.float32, tag="agg")
                        nc.vector.tensor_tensor(out=agg[:], in0=slabA, in1=slabB, op=ALU.add)
                        if kstage == "g":
                            nc.sync.dma_start(out=outT[:, blk], in_=agg[:])
                            continue
                        mean = wp.tile([P, P], dt.bfloat16, tag="mean")
                        nc.vector.tensor_tensor(out=mean[:], in0=agg[:], in1=invT_sb[:, blk], op=ALU.mult)
                        ps = psp.tile([P, P], dt.float32)
                        nc.tensor.matmul(out=ps[:], lhsT=wl_sb[:], rhs=mean[:], start=True, stop=False)
                        nc.tensor.matmul(out=ps[:], lhsT=wr_sb[:], rhs=rhs2[:, blk], start=False, stop=True)
                        if kstage == "m":
                            dbg = wp.tile([P, P], dt.float32, tag="o")
                            nc.vector.tensor_copy(dbg[:], ps[:])
                            nc.sync.dma_start(out=outT[:, blk], in_=dbg[:])
                            continue
                        if L == 0:
                            t1 = wp.tile([P, P], dt.bfloat16, tag="t1")
                            nc.scalar.activation(out=t1[:], in_=ps[:], func=AF.Relu, bias=b1_sb[:, 0:1], scale=1.0)
                            if kstage == "a":
                                dbg = wp.tile([P, P], dt.float32, tag="o")
                                nc.vector.tensor_copy(dbg[:], t1[:])
                                nc.sync.dma_start(out=outT[:, blk], in_=dbg[:])
                                continue
                            nc.vector.tensor_tensor(out=hT_sb[:, blk], in0=t1[:], in1=m2T_sb[:, blk], op=ALU.mult)
                            if stage < 2:
                                dbg = wp.tile([P, P], dt.float32, tag="o")
                                nc.vector.tensor_copy(dbg[:], hT_sb[:, blk])
                                nc.sync.dma_start(out=outT[:, blk], in_=dbg[:])
                                continue
                            tp = psq.tile([P, P], dt.bfloat16)
                            nc.tensor.transpose(out=tp[:], in_=hT_sb[:, blk], identity=ident[:])
                            hr = wp.tile([P, P], dt.bfloat16, tag="hr")
                            nc.vector.tensor_copy(hr[:], tp[:])
                            nc.sync.dma_start(out=cc_in[blk, :], in_=hr[:])
                            if stage == 2:
                                dbg = wp.tile([P, P], dt.float32, tag="o")
                                nc.vector.tensor_copy(dbg[:], hT_sb[:, blk])
                                nc.sync.dma_start(out=outT[:, blk], in_=dbg[:])
                        else:
                            o = wp.tile([P, P], dt.float32, tag="o")
                            nc.vector.tensor_tensor(
                                out=o[:], in0=ps[:],
                                in1=b2_sb[:, 0:1].to_broadcast([P, P]), op=ALU.add,
                            )
                            nc.sync.dma_start(out=outT[:, blk], in_=o[:])
                if L == 0 and stage >= 3:
                    nc.gpsimd.collective_compute(
                        "AllGather",
                        mybir.AluOpType.bypass,
                        ins=[cc_in[:, :]],
                        outs=[h_full[:, :]],
                        replica_groups=[list(range(ncores))],
                    )
    nc.compile()
    return nc


# --------------------------------------------------------------------------
# Input map construction + host post-processing
# --------------------------------------------------------------------------

def make_in_maps(meta, x, mask, W1_l, b1_l, W1_r, W2_l, b2_l, W2_r):
    m = meta
    N, ncores, npad, npc = m["N"], m["ncores"], m["npad"], m["npc"]
    s = m["split1"]
    zrow = np.zeros((1, D), BF16)
    xb = np.asarray(x, np.float32).astype(BF16)
    xtabA = np.ascontiguousarray(np.concatenate([xb[:s], zrow]))
    xtabB = np.ascontiguousarray(np.concatenate([xb[s:], zrow]))
    w1lb = np.ascontiguousarray(np.asarray(W1_l, np.float32).astype(BF16))
    w1rb = np.ascontiguousarray(np.asarray(W1_r, np.float32).astype(BF16))
    w2lb = np.ascontiguousarray(np.asarray(W2_l, np.float32).astype(BF16))
    w2rb = np.ascontiguousarray(np.asarray(W2_r, np.float32).astype(BF16))
    b1c = np.ascontiguousarray(np.asarray(b1_l, np.float32).reshape(P, 1))
    b2c = np.ascontiguousarray(np.asarray(b2_l, np.float32).reshape(P, 1))
    mask2 = np.asarray(mask, np.float32) * 2.0

    maps = []
    for c in range(ncores):
        ids = m["perm"][c]
        valid = ids >= 0
        safe = np.where(valid, ids, 0)
        xp = xb[safe]
        xp[~valid] = 0
        mp = mask2[safe].astype(BF16)
        mp[~valid] = 0
        inv = m["invcnt"][safe].copy()
        inv[~valid] = 1.0
        maps.append({
            "xtabA": xtabA, "xtabB": xtabB,
            "idx1": m["idx"][0][c], "idx2": m["idx"][1][c],
            "xT": np.ascontiguousarray(xp.T),
            "m2T": np.ascontiguousarray(mp.T),
            "invT": np.ascontiguousarray(np.broadcast_to(inv.reshape(1, npad), (P, npad))),
            "w1l": w1lb, "w1r": w1rb, "w2l": w2lb, "w2r": w2rb,
            "b1": b1c, "b2": b2c,
        })
    return maps


def assemble_output(meta, results):
    m = meta
    out = np.empty((m["N"], D), np.float32)
    for c in range(m["ncores"]):
        oT = np.asarray(results[c]["outT"], np.float32)
        ids = m["perm"][c][:m["npc"]]
        out[ids] = oT.T[:m["npc"]]
    return out


# --------------------------------------------------------------------------
# Entry point
# --------------------------------------------------------------------------

def _ensure_ntff_hook():
    """Reconstruct the axon NTFF profile hook if the image lacks
    antenv.axon_hooks (degraded boot). Needed only for trace=True."""
    import types
    try:
        from antenv.axon_hooks import get_axon_ntff_profile_hook
        if get_axon_ntff_profile_hook() is not None:
            return
    except ImportError:
        mod = types.ModuleType("antenv.axon_hooks")
        holder = [None]
        mod.set_axon_ntff_profile_hook = lambda h: holder.__setitem__(0, h)
        mod.get_axon_ntff_profile_hook = lambda: holder[0]
        sys.modules["antenv.axon_hooks"] = mod
        import antenv
        antenv.axon_hooks = mod
    if "/root/.axon_site" not in sys.path:
        sys.path.insert(0, "/root/.axon_site")
    from trn_agent_boot.trn_boot import _ntff_profile_via_ctypes
    from antenv.axon_hooks import set_axon_ntff_profile_hook
    hook = _ntff_profile_via_ctypes("/opt/axon/libaxon_pjrt.so")
    set_axon_ntff_profile_hook(hook)


_CACHE = {}


def _get_ctx(edge_index, N, ncores=8):
    ei = np.asarray(edge_index, np.int64)
    key = (N, ncores, hashlib.sha1(ei.tobytes()).hexdigest())
    ctx = _CACHE.get(key)
    if ctx is None:
        meta = build_meta(ei[0], ei[1], N, ncores)
        nc = build_nc(meta)
        _CACHE.clear()
        _CACHE[key] = ctx = (meta, nc)
    return ctx


def kernel(x, edge_index, drop_mask, W1_l, b1_l, W1_r, W2_l, b2_l, W2_r,
           trace=False):
    x = np.asarray(x, np.float32)
    meta, nc = _get_ctx(edge_index, x.shape[0])
    in_maps = make_in_maps(meta, x, drop_mask, W1_l, b1_l, W1_r, W2_l, b2_l, W2_r)
    if trace:
        _ensure_ntff_hook()
    from concourse.bass_utils import run_bass_kernel_spmd
    res = run_bass_kernel_spmd(
        nc, in_maps, core_ids=list(range(meta["ncores"])), trace=trace,
    )
    out = assemble_output(meta, res.results)
    if trace:
        return out, res
    return out


# revision 13
# speedup vs baseline: 1.4756x; 1.4756x over previous
"""2-layer GraphSAGE (mean) over 8 TRN2 NeuronCores.

Strategy:
  - Destination-shard nodes across 8 cores (6250/core). x is replicated into
    every core's DRAM at input-load time, so layer 1 needs no communication.
  - Host (numpy, uncounted) builds per-core padded-CSR gather schedules:
    nodes within a core are sorted by degree and packed into 128-node blocks;
    each block's neighbor lists are padded to the block max (K) and split by
    source-id half so every dma_gather table stays < 32768 rows (int16 idxs).
  - Device per block: dma_gather (transpose mode, bf16) pulls neighbor rows
    as columns [feat(p), slot]; DVE tree-adds fold K slots -> agg; multiply
    by 1/deg; two PE matmuls (mean@W_l + x@W_r) accumulate in PSUM; ACT does
    bias+relu; DVE applies the dropout mask. h rows go to DRAM via a PE
    transpose.
  - One AllGather exchanges h slices between layers; layer 2 gathers from the
    gathered table and writes the output transposed (host untransposes).
"""

import sys

for _p in ("/opt/trn_rl_repo",):
    if _p not in sys.path:
        sys.path.insert(0, _p)

import hashlib
import numpy as np
import ml_dtypes

BF16 = ml_dtypes.bfloat16
P = 128
D = 128


# --------------------------------------------------------------------------
# Host-side schedule construction
# --------------------------------------------------------------------------

def build_meta(src, dst, N, ncores, slot_budget=32):
    """Build the shared (SPMD) gather schedule + per-core index arrays."""
    src = np.asarray(src, np.int64)
    dst = np.asarray(dst, np.int64)
    assert ncores % 2 == 0
    ncg = ncores // 2  # cores per half-group
    split1 = N // 2
    # global (-degA,-degB) sort per half; deal 128-blocks round-robin to the
    # half's cores so every core's block b has a near-identical degree
    # profile (tight shared-max K). One extra all-pad block per core
    # guarantees a zero sentinel row on every core.
    deg = np.bincount(dst, minlength=N)
    invcnt = (1.0 / np.maximum(deg, 1.0)).astype(np.float32)
    degA = np.bincount(dst[src < split1], minlength=N)
    degB = deg - degA

    gblocks_per_half = -(-split1 // P) + ncg  # incl. >=1 all-pad block/core
    gblocks_per_half = -(-gblocks_per_half // ncg) * ncg
    blocks = gblocks_per_half // ncg
    npad = blocks * P
    split2 = ncg * npad
    assert split1 + 1 <= 32768 and (N - split1) + 1 <= 32768
    assert split2 <= 32768

    perm = -np.ones((ncores, npad), np.int64)
    rank = np.empty(N, np.int64)
    core_of = np.empty(N, np.int64)
    for grp in range(2):
        ids = np.arange(0, split1) if grp == 0 else np.arange(split1, N)
        cores = range(grp * ncg, (grp + 1) * ncg)
        order = np.lexsort((-degB[ids], -degA[ids]))
        sids = ids[order]
        padded = -np.ones(gblocks_per_half * P, np.int64)
        padded[:len(sids)] = sids
        gb = padded.reshape(-1, P)
        for j, c in enumerate(cores):
            mine = gb[j::ncg].reshape(-1)
            perm[c] = mine
            valid = mine >= 0
            rank[mine[valid]] = np.nonzero(valid)[0]
            core_of[mine[valid]] = c
        assert all(perm[c][npad - 1] == -1 for c in cores)
    pos = core_of * npad + rank  # position in the allgathered h table

    # per (layer, half, core): edge placement (block, lane, k, local idx)
    Ks = np.zeros((2, 2, ncores, blocks), np.int64)
    placed = {}
    dcore = core_of[dst]
    for c in range(ncores):
        sel = dcore == c
        s_c = src[sel]
        r_all = rank[dst[sel]]
        for L in range(2):
            key = s_c if L == 0 else pos[s_c]
            spl = split1 if L == 0 else split2
            half = (key >= spl).astype(np.int64)
            li = np.where(half == 0, key, key - spl)
            for h in (0, 1):
                m2 = half == h
                rr = r_all[m2]
                ll = li[m2]
                o = np.argsort(rr, kind="stable")
                rr = rr[o]
                ll = ll[o]
                cnts = np.bincount(rr, minlength=npad)
                first = np.concatenate([[0], np.cumsum(cnts)])[:-1]
                k = np.arange(len(rr)) - first[rr]
                blk = rr // P
                lane = rr % P
                Kblk = np.zeros(blocks, np.int64)
                if len(rr):
                    np.maximum.at(Kblk, blk, k + 1)
                placed[(L, h, c)] = (blk, lane, k, ll)
                Ks[L, h, c] = Kblk

    K = np.maximum(Ks.max(axis=2), 1)  # [layer][half][block], shared schedule

    layers = []
    idx_arrays = [[None] * ncores, [None] * ncores]
    for L in range(2):
        KA, KB = K[L, 0], K[L, 1]
        # greedy grouping of blocks under the per-half slot budget
        groups_blocks = []
        cur, curA, curB = [], 0, 0
        for b in range(blocks):
            if cur and (curA + KA[b] > slot_budget or curB + KB[b] > slot_budget):
                groups_blocks.append(cur)
                cur, curA, curB = [], 0, 0
            cur.append(b)
            curA += KA[b]
            curB += KB[b]
        groups_blocks.append(cur)

        sentA = split1 if L == 0 else (npad - 1)
        sentB = (N - split1) if L == 0 else (npad - 1)

        # assembly layout: per group, all A segments then all B segments
        total = int((KA.sum() + KB.sum()) * P)
        baseA = np.zeros(blocks, np.int64)  # elem offset of block b's A segment
        baseB = np.zeros(blocks, np.int64)
        groups = []
        off = 0
        for g in groups_blocks:
            ginfo = {"blocks": []}
            a0 = off
            for b in g:
                baseA[b] = off
                off += int(KA[b]) * P
            ginfo["colA"] = a0 // 16
            ginfo["NIA"] = off - a0
            b0 = off
            for b in g:
                baseB[b] = off
                off += int(KB[b]) * P
            ginfo["colB"] = b0 // 16
            ginfo["NIB"] = off - b0
            oa = 0
            ob = 0
            for b in g:
                ginfo["blocks"].append(
                    (b, oa, int(KA[b]), ob, int(KB[b]))
                )
                oa += int(KA[b])
                ob += int(KB[b])
            groups.append(ginfo)
        assert off == total

        for c in range(ncores):
            flat = np.empty(total, np.int16)
            # default sentinels
            for g in groups:
                a0 = g["colA"] * 16
                flat[a0:a0 + g["NIA"]] = sentA
                b0 = g["colB"] * 16
                flat[b0:b0 + g["NIB"]] = sentB
            for h, base in ((0, baseA), (1, baseB)):
                blk, lane, k, ll = placed[(L, h, c)]
                if len(blk):
                    np.add.at  # noqa (no-op; keep linters quiet)
                    posn = base[blk] + k * P + lane
                    flat[posn] = ll.astype(np.int16)
            idx_arrays[L][c] = np.ascontiguousarray(np.tile(flat.reshape(-1, 16).T, (8, 1)))

        layers.append({"groups": groups, "C": total // 16})

    return {
        "N": N, "ncores": ncores, "blocks": blocks, "npad": npad,
        "split1": split1, "split2": split2,
        "perm": perm, "invcnt": invcnt,
        "layers": layers, "idx": idx_arrays,
        "tabA1": split1 + 1, "tabB1": (N - split1) + 1,
    }


# --------------------------------------------------------------------------
# Bass graph
# --------------------------------------------------------------------------

def build_nc(meta):
    import os
    kstage = os.environ.get("KSTAGE", "5")
    stage = {"g": 1, "m": 1, "a": 1}.get(kstage, int(kstage) if kstage.isdigit() else 5)
    from concourse import bacc, mybir
    from concourse.tile import TileContext
    from concourse.masks import make_identity

    dt = mybir.dt
    ALU = mybir.AluOpType
    AF = mybir.ActivationFunctionType
    m = meta
    npad, ncores, blocks = m["npad"], m["ncores"], m["blocks"]

    nc = bacc.Bacc()

    xtabA = nc.declare_dram_parameter("xtabA", [m["tabA1"], D], dt.bfloat16, isOutput=False)
    xtabB = nc.declare_dram_parameter("xtabB", [m["tabB1"], D], dt.bfloat16, isOutput=False)
    idx1 = nc.declare_dram_parameter("idx1", [P, m["layers"][0]["C"]], dt.int16, isOutput=False)
    idx2 = nc.declare_dram_parameter("idx2", [P, m["layers"][1]["C"]], dt.int16, isOutput=False)
    xT = nc.declare_dram_parameter("xT", [P, npad], dt.bfloat16, isOutput=False)
    m2T = nc.declare_dram_parameter("m2T", [P, npad], dt.bfloat16, isOutput=False)
    invT = nc.declare_dram_parameter("invT", [P, npad], dt.float32, isOutput=False)
    w1l = nc.declare_dram_parameter("w1l", [P, P], dt.bfloat16, isOutput=False)
    w1r = nc.declare_dram_parameter("w1r", [P, P], dt.bfloat16, isOutput=False)
    w2l = nc.declare_dram_parameter("w2l", [P, P], dt.bfloat16, isOutput=False)
    w2r = nc.declare_dram_parameter("w2r", [P, P], dt.bfloat16, isOutput=False)
    b1 = nc.declare_dram_parameter("b1", [P, 1], dt.float32, isOutput=False)
    b2 = nc.declare_dram_parameter("b2", [P, 1], dt.float32, isOutput=False)
    outT = nc.declare_dram_parameter("outT", [P, npad], dt.float32, isOutput=True)

    cc_in = nc.dram_tensor("cc_in", [npad, D], dt.bfloat16)
    h_full = nc.dram_tensor("h_full", [ncores * npad, D], dt.bfloat16, addr_space="Shared")

    def tree(gt, off, K):
        """Fold K slots at slot-offset `off` of gather tile gt down to 1."""
        while K > 1:
            h = K // 2
            a = K - h
            dstap = gt[:, (off) * P:(off + h) * P]
            srcap = gt[:, (off + a) * P:(off + a + h) * P]
            nc.vector.tensor_tensor(out=dstap, in0=dstap, in1=srcap, op=ALU.add)
            K = a
        return gt[:, off * P:(off + 1) * P]

    with TileContext(nc) as tc:
        with (
            tc.tile_pool(name="persist", bufs=1) as pers,
            tc.tile_pool(name="work", bufs=4) as wp,
            tc.tile_pool(name="gath", bufs=2) as gp,
            tc.tile_pool(name="ps", bufs=2, space="PSUM") as psp,
            tc.tile_pool(name="pst", bufs=2, space="PSUM") as psq,
        ):
            def load(dram, shape, dtype, tag):
                t = pers.tile(shape, dtype, tag=tag)
                nc.sync.dma_start(out=t[:], in_=dram[:])
                return t

            idx_sb = [
                load(idx1, [P, m["layers"][0]["C"]], dt.int16, "idx1"),
                load(idx2, [P, m["layers"][1]["C"]], dt.int16, "idx2"),
            ]
            xT_sb = load(xT, [P, npad], dt.bfloat16, "xT")
            m2T_sb = load(m2T, [P, npad], dt.bfloat16, "m2T")
            invT_sb = load(invT, [P, npad], dt.float32, "invT")
            w_sb = [
                (load(w1l, [P, P], dt.bfloat16, "w1l"), load(w1r, [P, P], dt.bfloat16, "w1r")),
                (load(w2l, [P, P], dt.bfloat16, "w2l"), load(w2r, [P, P], dt.bfloat16, "w2r")),
            ]
            b1_sb = load(b1, [P, 1], dt.float32, "b1")
            b2_sb = load(b2, [P, 1], dt.float32, "b2")
            ident = pers.tile([P, P], dt.bfloat16, tag="ident")
            make_identity(nc, ident[:])
            hT_sb = pers.tile([P, npad], dt.bfloat16, tag="hT")

            for L in range(2 if stage >= 4 else 1):
                lm = m["layers"][L]
                if L == 0:
                    tabs = (xtabA[:, :], xtabB[:, :])
                else:
                    tabs = (h_full[0:m["split2"], :], h_full[m["split2"]:, :])
                wl_sb, wr_sb = w_sb[L]
                rhs2 = xT_sb if L == 0 else hT_sb
                CALL_SLOTS = 7  # 7*128=896 idxs -> 56 descs, one SWDGE packet
                for g in lm["groups"]:
                    gts = []
                    for tab, col, NI in (
                        (tabs[0], g["colA"], g["NIA"]),
                        (tabs[1], g["colB"], g["NIB"]),
                    ):
                        t = gp.tile([P, NI], dt.bfloat16, tag=f"g{len(gts)}")
                        for s0 in range(0, NI, CALL_SLOTS * P):
                            ni = min(CALL_SLOTS * P, NI - s0)
                            nc.gpsimd.dma_gather(
                                out_ap=t[:, s0:s0 + ni].rearrange("p (o n) -> p o n", o=1),
                                in_ap=tab,
                                idxs_ap=idx_sb[L][:, (col + s0 // 16):(col + (s0 + ni) // 16)],
                                num_idxs=ni,
                                num_idxs_reg=ni,
                                elem_size=D,
                                transpose=True,
                                single_packet=True,
                            )
                        gts.append(t)
                    for b, oA, KA, oB, KB in g["blocks"]:
                        blk = slice(b * P, (b + 1) * P)
                        slabA = tree(gts[0], oA, KA)
                        slabB = tree(gts[1], oB, KB)
                        agg = wp.tile([P, P], dt.float32, tag="agg")
                        nc.vector.tensor_tensor(out=agg[:], in0=slabA, in1=slabB, op=ALU.add)
                        if kstage == "g":
                            nc.sync.dma_start(out=outT[:, blk], in_=agg[:])
                            continue
                        mean = wp.tile([P, P], dt.bfloat16, tag="mean")
                        nc.vector.tensor_tensor(out=mean[:], in0=agg[:], in1=invT_sb[:, blk], op=ALU.mult)
                        ps = psp.tile([P, P], dt.float32)
                        nc.tensor.matmul(out=ps[:], lhsT=wl_sb[:], rhs=mean[:], start=True, stop=False)
                        nc.tensor.matmul(out=ps[:], lhsT=wr_sb[:], rhs=rhs2[:, blk], start=False, stop=True)
                        if kstage == "m":
                            dbg = wp.tile([P, P], dt.float32, tag="o")
                            nc.vector.tensor_copy(dbg[:], ps[:])
                            nc.sync.dma_start(out=outT[:, blk], in_=dbg[:])
                            continue
                        if L == 0:
                            t1 = wp.tile([P, P], dt.bfloat16, tag="t1")
                            nc.scalar.activation(out=t1[:], in_=ps[:], func=AF.Relu, bias=b1_sb[:, 0:1], scale=1.0)
                            if kstage == "a":
                                dbg = wp.tile([P, P], dt.float32, tag="o")
                                nc.vector.tensor_copy(dbg[:], t1[:])
                                nc.sync.dma_start(out=outT[:, blk], in_=dbg[:])
                                continue
                            nc.vector.tensor_tensor(out=hT_sb[:, blk], in0=t1[:], in1=m2T_sb[:, blk], op=ALU.mult)
                            if stage < 2:
                                dbg = wp.tile([P, P], dt.float32, tag="o")
                                nc.vector.tensor_copy(dbg[:], hT_sb[:, blk])
                                nc.sync.dma_start(out=outT[:, blk], in_=dbg[:])
                                continue
                            tp = psq.tile([P, P], dt.bfloat16)
                            nc.tensor.transpose(out=tp[:], in_=hT_sb[:, blk], identity=ident[:])
                            hr = wp.tile([P, P], dt.bfloat16, tag="hr")
                            nc.vector.tensor_copy(hr[:], tp[:])
                            nc.sync.dma_start(out=cc_in[blk, :], in_=hr[:])
                            if stage == 2:
                                dbg = wp.tile([P, P], dt.float32, tag="o")
                                nc.vector.tensor_copy(dbg[:], hT_sb[:, blk])
                                nc.sync.dma_start(out=outT[:, blk], in_=dbg[:])
                        else:
                            o = wp.tile([P, P], dt.float32, tag="o")
                            nc.vector.tensor_tensor(
                                out=o[:], in0=ps[:],
                                in1=b2_sb[:, 0:1].to_broadcast([P, P]), op=ALU.add,
                            )
                            nc.sync.dma_start(out=outT[:, blk], in_=o[:])
                if L == 0 and stage >= 3:
                    nc.gpsimd.collective_compute(
                        "AllGather",
                        mybir.AluOpType.bypass,
                        ins=[cc_in[:, :]],
                        outs=[h_full[:, :]],
                        replica_groups=[list(range(ncores))],
                    )
    nc.compile()
    return nc


# --------------------------------------------------------------------------
# Input map construction + host post-processing
# --------------------------------------------------------------------------

def make_in_maps(meta, x, mask, W1_l, b1_l, W1_r, W2_l, b2_l, W2_r):
    m = meta
    N, ncores, npad = m["N"], m["ncores"], m["npad"]
    s = m["split1"]
    zrow = np.zeros((1, D), BF16)
    xb = np.asarray(x, np.float32).astype(BF16)
    xtabA = np.ascontiguousarray(np.concatenate([xb[:s], zrow]))
    xtabB = np.ascontiguousarray(np.concatenate([xb[s:], zrow]))
    w1lb = np.ascontiguousarray(np.asarray(W1_l, np.float32).astype(BF16))
    w1rb = np.ascontiguousarray(np.asarray(W1_r, np.float32).astype(BF16))
    w2lb = np.ascontiguousarray(np.asarray(W2_l, np.float32).astype(BF16))
    w2rb = np.ascontiguousarray(np.asarray(W2_r, np.float32).astype(BF16))
    b1c = np.ascontiguousarray(np.asarray(b1_l, np.float32).reshape(P, 1))
    b2c = np.ascontiguousarray(np.asarray(b2_l, np.float32).reshape(P, 1))
    mask2 = np.asarray(mask, np.float32) * 2.0

    maps = []
    for c in range(ncores):
        ids = m["perm"][c]
        valid = ids >= 0
        safe = np.where(valid, ids, 0)
        xp = xb[safe]
        xp[~valid] = 0
        mp = mask2[safe].astype(BF16)
        mp[~valid] = 0
        inv = m["invcnt"][safe].copy()
        inv[~valid] = 1.0
        maps.append({
            "xtabA": xtabA, "xtabB": xtabB,
            "idx1": m["idx"][0][c], "idx2": m["idx"][1][c],
            "xT": np.ascontiguousarray(xp.T),
            "m2T": np.ascontiguousarray(mp.T),
            "invT": np.ascontiguousarray(np.broadcast_to(inv.reshape(1, npad), (P, npad))),
            "w1l": w1lb, "w1r": w1rb, "w2l": w2lb, "w2r": w2rb,
            "b1": b1c, "b2": b2c,
        })
    return maps


def assemble_output(meta, results):
    m = meta
    out = np.empty((m["N"], D), np.float32)
    for c in range(m["ncores"]):
        oT = np.asarray(results[c]["outT"], np.float32)
        ids = m["perm"][c]
        valid = ids >= 0
        out[ids[valid]] = oT.T[valid]
    return out


# --------------------------------------------------------------------------
# Entry point
# --------------------------------------------------------------------------

def _ensure_ntff_hook():
    """Reconstruct the axon NTFF profile hook if the image lacks
    antenv.axon_hooks (degraded boot). Needed only for trace=True."""
    import types
    try:
        from antenv.axon_hooks import get_axon_ntff_profile_hook
        if get_axon_ntff_profile_hook() is not None:
            return
    except ImportError:
        mod = types.ModuleType("antenv.axon_hooks")
        holder = [None]
        mod.set_axon_ntff_profile_hook = lambda h: holder.__setitem__(0, h)
        mod.get_axon_ntff_profile_hook = lambda: holder[0]
        sys.modules["antenv.axon_hooks"] = mod
        import antenv
        antenv.axon_hooks = mod
    if "/root/.axon_site" not in sys.path:
        sys.path.insert(0, "/root/.axon_site")
    from trn_agent_boot.trn_boot import _ntff_profile_via_ctypes
    from antenv.axon_hooks import set_axon_ntff_profile_hook
    hook = _ntff_profile_via_ctypes("/opt/axon/libaxon_pjrt.so")
    set_axon_ntff_profile_hook(hook)


_CACHE = {}


def _get_ctx(edge_index, N, ncores=8):
    ei = np.asarray(edge_index, np.int64)
    key = (N, ncores, hashlib.sha1(ei.tobytes()).hexdigest())
    ctx = _CACHE.get(key)
    if ctx is None:
        meta = build_meta(ei[0], ei[1], N, ncores)
        nc = build_nc(meta)
        _CACHE.clear()
        _CACHE[key] = ctx = (meta, nc)
    return ctx


def kernel(x, edge_index, drop_mask, W1_l, b1_l, W1_r, W2_l, b2_l, W2_r,
           trace=False):
    x = np.asarray(x, np.float32)
    meta, nc = _get_ctx(edge_index, x.shape[0])
    in_maps = make_in_maps(meta, x, drop_mask, W1_l, b1_l, W1_r, W2_l, b2_l, W2_r)
    if trace:
        _ensure_ntff_hook()
    from concourse.bass_utils import run_bass_kernel_spmd
    res = run_bass_kernel_spmd(
        nc, in_maps, core_ids=list(range(meta["ncores"])), trace=trace,
    )
    out = assemble_output(meta, res.results)
    if trace:
        return out, res
    return out


# revision 14
# speedup vs baseline: 1.9163x; 1.2986x over previous
"""2-layer GraphSAGE (mean) over 8 TRN2 NeuronCores — one-hot PE scatter design.

  - Destination-shard nodes across 8 cores (contiguous 6250-node ranges). x is
    replicated into every core's DRAM at input-load time; layer 1 needs no
    communication.
  - Host (numpy, uncounted) sorts each core's edges by destination window
    (128 nodes) and source half (tables must stay <32768 rows for int16
    dma_gather indices), padding each (window, half) run to a multiple of 128
    with zero-row sentinels (~6% overhead).
  - Device: dma_gather (bf16, non-transpose) pulls source rows in big batched
    calls -> [128 edge-lanes, slab, 128 feat] tiles. Per 128-edge chunk, DVE
    builds a one-hot [edge, dst-lane] via is_equal against a constant iota,
    and PE matmul-accumulates agg[dst, feat] in PSUM across the window's
    chunks. Mean = agg * 1/deg (DVE, PSUM read). PE transposes mean, then
    out_rows = mean @ W_l + x @ W_r + 1·b (three matmuls, bias as K=1 rank-1
    update). ACT applies relu; DVE applies the dropout mask.
  - One AllGather exchanges h rows between layers; layer 2 repeats with
    tables sliced from the gathered h.
"""

import sys

for _p in ("/opt/trn_rl_repo",):
    if _p not in sys.path:
        sys.path.insert(0, _p)

import hashlib
import numpy as np
import ml_dtypes

BF16 = ml_dtypes.bfloat16
P = 128
D = 128

SLAB_BUDGET = 32  # max 128-edge chunks per gather call (4096 idxs, 1 MB bf16)


# --------------------------------------------------------------------------
# Host-side schedule construction
# --------------------------------------------------------------------------

def build_meta(src, dst, N, ncores):
    src = np.asarray(src, np.int64)
    dst = np.asarray(dst, np.int64)
    npc = N // ncores
    assert npc * ncores == N, (N, ncores)
    blocks = -(-npc // P)
    npad = blocks * P
    assert npc < npad, "need pad ranks for layer-2 zero sentinels"
    assert ncores % 2 == 0
    split1 = N // 2
    split2 = (ncores // 2) * npad
    assert split1 + 1 <= 32768 and (N - split1) + 1 <= 32768
    assert split2 <= 32768

    deg = np.bincount(dst, minlength=N)
    invcnt = (1.0 / np.maximum(deg, 1.0)).astype(np.float32)

    perm = -np.ones((ncores, npad), np.int64)
    for c in range(ncores):
        perm[c, :npc] = np.arange(c * npc, (c + 1) * npc)
    rank = np.mod(np.arange(N), npc)
    core_of = np.arange(N) // npc
    pos = core_of * npad + rank  # row in the allgathered h table

    layers = []
    idx_arrays = [[None] * ncores, [None] * ncores]
    dstl_arrays = [[None] * ncores, [None] * ncores]
    for L in range(2):
        spl = split1 if L == 0 else split2
        sentA = split1 if L == 0 else npc
        sentB = (N - split1) if L == 0 else npc

        # per-core edge placements + shared chunk schedule (max over cores)
        nch = np.zeros((blocks, 2), np.int64)
        per_core = []
        for c in range(ncores):
            sel = core_of[dst] == c
            s_c = src[sel]
            r = rank[dst[sel]]
            key = s_c if L == 0 else pos[s_c]
            half = (key >= spl).astype(np.int64)
            li = np.where(half == 0, key, key - spl)
            w = r // P
            dl = r % P
            cnt = np.zeros((blocks, 2), np.int64)
            np.add.at(cnt, (w, half), 1)
            np.maximum(nch, -(-cnt // P), out=nch)
            per_core.append((w, half, li, dl))
        nch = np.maximum(nch, 1)  # at least one chunk per (window, half)

        # group windows so each half's slab count stays under budget
        groups_w = []
        cur, ca, cb = [], 0, 0
        for b in range(blocks):
            a, bb = int(nch[b, 0]), int(nch[b, 1])
            if cur and (ca + a > SLAB_BUDGET or cb + bb > SLAB_BUDGET):
                groups_w.append(cur)
                cur, ca, cb = [], 0, 0
            cur.append(b)
            ca += a
            cb += bb
        groups_w.append(cur)

        # slot layout: per group, all A slabs (window-major) then all B slabs
        total_slabs = int(nch.sum())
        slab_of = np.zeros((blocks, 2), np.int64)  # first global slab of (w,h)
        groups = []
        off = 0
        for gw in groups_w:
            ginfo = {"windows": []}
            for h, hn in ((0, "A"), (1, "B")):
                g0 = off
                for b in gw:
                    slab_of[b, h] = off
                    off += int(nch[b, h])
                ginfo["col" + hn] = g0 * 8  # idx column: slab*128/16
                ginfo["NI" + hn] = (off - g0) * P
                ginfo["base" + hn] = g0
            for b in gw:
                chunks = []
                for h, hn in ((0, "A"), (1, "B")):
                    for j in range(int(nch[b, h])):
                        gslab = int(slab_of[b, h]) + j
                        chunks.append((h, gslab - ginfo["base" + hn], gslab))
                ginfo["windows"].append({"w": b, "chunks": chunks})
            groups.append(ginfo)
        assert off == total_slabs

        # fill per-core idx + dstl arrays
        for c in range(ncores):
            w, half, li, dl = per_core[c]
            flat = np.empty(total_slabs * P, np.int16)
            dflat = np.zeros(total_slabs * P, np.int16)
            for b in range(blocks):
                for h, sent in ((0, sentA), (1, sentB)):
                    s0 = int(slab_of[b, h]) * P
                    flat[s0:s0 + int(nch[b, h]) * P] = sent
            grp = w * 2 + half
            order = np.argsort(grp, kind="stable")
            w_s, half_s, li_s, dl_s = w[order], half[order], li[order], dl[order]
            cnts = np.bincount(grp, minlength=blocks * 2)
            first = np.concatenate([[0], np.cumsum(cnts)])[:-1]
            k = np.arange(len(w_s)) - first[grp[order]]
            posn = slab_of[w_s, half_s] * P + k
            flat[posn] = li_s.astype(np.int16)
            dflat[posn] = dl_s.astype(np.int16)
            idx_arrays[L][c] = np.ascontiguousarray(
                np.tile(flat.reshape(-1, 16).T, (8, 1)))
            dstl_arrays[L][c] = np.ascontiguousarray(
                dflat.reshape(total_slabs, P).T.astype(BF16))

        layers.append({"groups": groups, "C": total_slabs * 8,
                       "TC": total_slabs})

    return {
        "N": N, "ncores": ncores, "npc": npc, "blocks": blocks, "npad": npad,
        "split1": split1, "split2": split2,
        "perm": perm, "invcnt": invcnt,
        "layers": layers, "idx": idx_arrays, "dstl": dstl_arrays,
        "tabA1": split1 + 1, "tabB1": (N - split1) + 1,
    }


# --------------------------------------------------------------------------
# Bass graph
# --------------------------------------------------------------------------

def build_nc(meta):
    from concourse import bacc, mybir
    from concourse.tile import TileContext
    from concourse.masks import make_identity

    dt = mybir.dt
    ALU = mybir.AluOpType
    AF = mybir.ActivationFunctionType
    m = meta
    npad, ncores, blocks = m["npad"], m["ncores"], m["blocks"]

    nc = bacc.Bacc()

    xtabA = nc.declare_dram_parameter("xtabA", [m["tabA1"], D], dt.bfloat16, isOutput=False)
    xtabB = nc.declare_dram_parameter("xtabB", [m["tabB1"], D], dt.bfloat16, isOutput=False)
    idx1 = nc.declare_dram_parameter("idx1", [P, m["layers"][0]["C"]], dt.int16, isOutput=False)
    idx2 = nc.declare_dram_parameter("idx2", [P, m["layers"][1]["C"]], dt.int16, isOutput=False)
    dstl1 = nc.declare_dram_parameter("dstl1", [P, m["layers"][0]["TC"]], dt.bfloat16, isOutput=False)
    dstl2 = nc.declare_dram_parameter("dstl2", [P, m["layers"][1]["TC"]], dt.bfloat16, isOutput=False)
    xT = nc.declare_dram_parameter("xT", [P, npad], dt.bfloat16, isOutput=False)
    m2r = nc.declare_dram_parameter("m2r", [npad, D], dt.bfloat16, isOutput=False)
    invc = nc.declare_dram_parameter("invc", [P, blocks], dt.float32, isOutput=False)
    iota = nc.declare_dram_parameter("iota", [P, P], dt.bfloat16, isOutput=False)
    onesr = nc.declare_dram_parameter("onesr", [1, P], dt.bfloat16, isOutput=False)
    w1l = nc.declare_dram_parameter("w1l", [P, P], dt.bfloat16, isOutput=False)
    w1r = nc.declare_dram_parameter("w1r", [P, P], dt.bfloat16, isOutput=False)
    w2l = nc.declare_dram_parameter("w2l", [P, P], dt.bfloat16, isOutput=False)
    w2r = nc.declare_dram_parameter("w2r", [P, P], dt.bfloat16, isOutput=False)
    b1r = nc.declare_dram_parameter("b1r", [1, P], dt.bfloat16, isOutput=False)
    b2r = nc.declare_dram_parameter("b2r", [1, P], dt.bfloat16, isOutput=False)
    out = nc.declare_dram_parameter("out", [npad, D], dt.float32, isOutput=True)

    cc_in = nc.dram_tensor("cc_in", [npad, D], dt.bfloat16)
    h_full = nc.dram_tensor("h_full", [ncores * npad, D], dt.bfloat16, addr_space="Shared")

    with TileContext(nc) as tc:
        with (
            tc.tile_pool(name="persist", bufs=1) as pers,
            tc.tile_pool(name="work", bufs=4) as wp,
            tc.tile_pool(name="oh", bufs=4) as ohp,
            tc.tile_pool(name="gath", bufs=2) as gp,
            tc.tile_pool(name="psagg", bufs=2, space="PSUM") as psa,
            tc.tile_pool(name="pstr", bufs=2, space="PSUM") as pst,
            tc.tile_pool(name="psout", bufs=2, space="PSUM") as pso,
        ):
            def load(dram, shape, dtype, tag):
                t = pers.tile(shape, dtype, tag=tag)
                nc.sync.dma_start(out=t[:], in_=dram[:])
                return t

            idx_sb = [
                load(idx1, [P, m["layers"][0]["C"]], dt.int16, "idx1"),
                load(idx2, [P, m["layers"][1]["C"]], dt.int16, "idx2"),
            ]
            dstl_sb = [
                load(dstl1, [P, m["layers"][0]["TC"]], dt.bfloat16, "dstl1"),
                load(dstl2, [P, m["layers"][1]["TC"]], dt.bfloat16, "dstl2"),
            ]
            xT_sb = load(xT, [P, npad], dt.bfloat16, "xT")
            invc_sb = load(invc, [P, blocks], dt.float32, "invc")
            iota_sb = load(iota, [P, P], dt.bfloat16, "iota")
            ones_sb = load(onesr, [1, P], dt.bfloat16, "ones")
            w_sb = [
                (load(w1l, [P, P], dt.bfloat16, "w1l"), load(w1r, [P, P], dt.bfloat16, "w1r"),
                 load(b1r, [1, P], dt.bfloat16, "b1r")),
                (load(w2l, [P, P], dt.bfloat16, "w2l"), load(w2r, [P, P], dt.bfloat16, "w2r"),
                 load(b2r, [1, P], dt.bfloat16, "b2r")),
            ]
            ident = pers.tile([P, P], dt.bfloat16, tag="ident")
            make_identity(nc, ident[:])
            hT_sb = pers.tile([P, npad], dt.bfloat16, tag="hT")

            for L in range(2):
                lm = m["layers"][L]
                if L == 0:
                    tabs = (xtabA[:, :], xtabB[:, :])
                else:
                    tabs = (h_full[0:m["split2"], :], h_full[m["split2"]:, :])
                wl_sb, wr_sb, br_sb = w_sb[L]
                side_sb = xT_sb if L == 0 else hT_sb
                for g in lm["groups"]:
                    gts = []
                    for tab, col, NI in (
                        (tabs[0], g["colA"], g["NIA"]),
                        (tabs[1], g["colB"], g["NIB"]),
                    ):
                        nslab = NI // P
                        t = gp.tile([P, max(nslab, 1), P], dt.bfloat16, tag=f"g{len(gts)}")
                        for s0 in range(0, nslab, SLAB_BUDGET):
                            sl = min(SLAB_BUDGET, nslab - s0)
                            nc.gpsimd.dma_gather(
                                out_ap=t[:, s0:s0 + sl, :],
                                in_ap=tab,
                                idxs_ap=idx_sb[L][:, col + s0 * 8: col + (s0 + sl) * 8],
                                num_idxs=sl * P,
                                num_idxs_reg=sl * P,
                                elem_size=D,
                                transpose=False,
                                single_packet=False,
                            )
                        gts.append(t)
                    for wi in g["windows"]:
                        b = wi["w"]
                        blk = slice(b * P, (b + 1) * P)
                        chunks = wi["chunks"]
                        ps = psa.tile([P, P], dt.float32)
                        for ci, (h, slab, t_g) in enumerate(chunks):
                            oh = ohp.tile([P, P], dt.bfloat16, tag="oh")
                            nc.vector.tensor_tensor(
                                out=oh[:],
                                in0=dstl_sb[L][:, t_g:t_g + 1].to_broadcast([P, P]),
                                in1=iota_sb[:],
                                op=ALU.is_equal,
                            )
                            nc.tensor.matmul(
                                out=ps[:], lhsT=oh[:], rhs=gts[h][:, slab, :],
                                start=(ci == 0), stop=(ci == len(chunks) - 1),
                            )
                        mean = wp.tile([P, P], dt.bfloat16, tag="mean")
                        nc.vector.tensor_tensor(
                            out=mean[:], in0=ps[:],
                            in1=invc_sb[:, b:b + 1].to_broadcast([P, P]),
                            op=ALU.mult,
                        )
                        tp = pst.tile([P, P], dt.bfloat16)
                        nc.tensor.transpose(out=tp[:], in_=mean[:], identity=ident[:])
                        meanT = wp.tile([P, P], dt.bfloat16, tag="meanT")
                        nc.vector.tensor_copy(meanT[:], tp[:])
                        po = pso.tile([P, P], dt.float32)
                        nc.tensor.matmul(out=po[:], lhsT=meanT[:], rhs=wl_sb[:], start=True, stop=False)
                        nc.tensor.matmul(out=po[:], lhsT=side_sb[:, blk], rhs=wr_sb[:], start=False, stop=False)
                        nc.tensor.matmul(out=po[:], lhsT=ones_sb[:], rhs=br_sb[:], start=False, stop=True)
                        if L == 0:
                            t1 = wp.tile([P, P], dt.bfloat16, tag="t1")
                            nc.scalar.activation(out=t1[:], in_=po[:], func=AF.Relu, bias=0.0, scale=1.0)
                            mk = wp.tile([P, P], dt.bfloat16, tag="mk")
                            nc.sync.dma_start(out=mk[:], in_=m2r[blk, :])
                            hr = wp.tile([P, P], dt.bfloat16, tag="hr")
                            nc.vector.tensor_tensor(out=hr[:], in0=t1[:], in1=mk[:], op=ALU.mult)
                            nc.sync.dma_start(out=cc_in[blk, :], in_=hr[:])
                            tp2 = pst.tile([P, P], dt.bfloat16)
                            nc.tensor.transpose(out=tp2[:], in_=hr[:], identity=ident[:])
                            nc.vector.tensor_copy(hT_sb[:, blk], tp2[:])
                        else:
                            o = wp.tile([P, P], dt.float32, tag="o")
                            nc.scalar.activation(out=o[:], in_=po[:], func=AF.Copy, bias=0.0, scale=1.0)
                            nc.sync.dma_start(out=out[blk, :], in_=o[:])
                if L == 0:
                    nc.gpsimd.collective_compute(
                        "AllGather",
                        mybir.AluOpType.bypass,
                        ins=[cc_in[:, :]],
                        outs=[h_full[:, :]],
                        replica_groups=[list(range(ncores))],
                    )
    nc.compile()
    return nc


# --------------------------------------------------------------------------
# Input map construction + host post-processing
# --------------------------------------------------------------------------

def make_in_maps(meta, x, mask, W1_l, b1_l, W1_r, W2_l, b2_l, W2_r):
    m = meta
    N, ncores, npad = m["N"], m["ncores"], m["npad"]
    s = m["split1"]
    zrow = np.zeros((1, D), BF16)
    xb = np.asarray(x, np.float32).astype(BF16)
    xtabA = np.ascontiguousarray(np.concatenate([xb[:s], zrow]))
    xtabB = np.ascontiguousarray(np.concatenate([xb[s:], zrow]))
    w1lb = np.ascontiguousarray(np.asarray(W1_l, np.float32).astype(BF16))
    w1rb = np.ascontiguousarray(np.asarray(W1_r, np.float32).astype(BF16))
    w2lb = np.ascontiguousarray(np.asarray(W2_l, np.float32).astype(BF16))
    w2rb = np.ascontiguousarray(np.asarray(W2_r, np.float32).astype(BF16))
    b1c = np.ascontiguousarray(np.asarray(b1_l, np.float32).astype(BF16).reshape(1, P))
    b2c = np.ascontiguousarray(np.asarray(b2_l, np.float32).astype(BF16).reshape(1, P))
    iota = np.ascontiguousarray(
        np.broadcast_to(np.arange(P, dtype=np.float32), (P, P)).astype(BF16))
    onesv = np.ones((1, P), BF16)
    mask2 = np.asarray(mask, np.float32) * 2.0

    maps = []
    for c in range(ncores):
        ids = m["perm"][c]
        valid = ids >= 0
        safe = np.where(valid, ids, 0)
        xp = xb[safe]
        xp[~valid] = 0
        mp = mask2[safe].astype(BF16)
        mp[~valid] = 0
        inv = m["invcnt"][safe].copy()
        inv[~valid] = 1.0
        maps.append({
            "xtabA": xtabA, "xtabB": xtabB,
            "idx1": m["idx"][0][c], "idx2": m["idx"][1][c],
            "dstl1": m["dstl"][0][c], "dstl2": m["dstl"][1][c],
            "xT": np.ascontiguousarray(xp.T),
            "m2r": np.ascontiguousarray(mp),
            "invc": np.ascontiguousarray(inv.reshape(m["blocks"], P).T),
            "iota": iota, "onesr": onesv,
            "w1l": w1lb, "w1r": w1rb, "w2l": w2lb, "w2r": w2rb,
            "b1r": b1c, "b2r": b2c,
        })
    return maps


def assemble_output(meta, results):
    m = meta
    out = np.empty((m["N"], D), np.float32)
    for c in range(m["ncores"]):
        o = np.asarray(results[c]["out"], np.float32)
        ids = m["perm"][c]
        valid = ids >= 0
        out[ids[valid]] = o[valid]
    return out


# --------------------------------------------------------------------------
# Entry point
# --------------------------------------------------------------------------

def _ensure_ntff_hook():
    """Reconstruct the axon NTFF profile hook if the image lacks
    antenv.axon_hooks (degraded boot). Needed only for trace=True."""
    import types
    try:
        from antenv.axon_hooks import get_axon_ntff_profile_hook
        if get_axon_ntff_profile_hook() is not None:
            return
    except ImportError:
        mod = types.ModuleType("antenv.axon_hooks")
        holder = [None]
        mod.set_axon_ntff_profile_hook = lambda h: holder.__setitem__(0, h)
        mod.get_axon_ntff_profile_hook = lambda: holder[0]
        sys.modules["antenv.axon_hooks"] = mod
        import antenv
        antenv.axon_hooks = mod
    if "/root/.axon_site" not in sys.path:
        sys.path.insert(0, "/root/.axon_site")
    from trn_agent_boot.trn_boot import _ntff_profile_via_ctypes
    from antenv.axon_hooks import set_axon_ntff_profile_hook
    hook = _ntff_profile_via_ctypes("/opt/axon/libaxon_pjrt.so")
    set_axon_ntff_profile_hook(hook)


_CACHE = {}


def _get_ctx(edge_index, N, ncores=8):
    ei = np.asarray(edge_index, np.int64)
    key = (N, ncores, hashlib.sha1(ei.tobytes()).hexdigest())
    ctx = _CACHE.get(key)
    if ctx is None:
        meta = build_meta(ei[0], ei[1], N, ncores)
        nc = build_nc(meta)
        _CACHE.clear()
        _CACHE[key] = ctx = (meta, nc)
    return ctx


def kernel(x, edge_index, drop_mask, W1_l, b1_l, W1_r, W2_l, b2_l, W2_r,
           trace=False):
    x = np.asarray(x, np.float32)
    meta, nc = _get_ctx(edge_index, x.shape[0])
    in_maps = make_in_maps(meta, x, drop_mask, W1_l, b1_l, W1_r, W2_l, b2_l, W2_r)
    if trace:
        _ensure_ntff_hook()
    from concourse.bass_utils import run_bass_kernel_spmd
    res = run_bass_kernel_spmd(
        nc, in_maps, core_ids=list(range(meta["ncores"])), trace=trace,
    )
    out = assemble_output(meta, res.results)
    if trace:
        return out, res
    return out
